# revision 19
# baseline (speedup 1.0000x reference)
"""Trainium2 Bass kernel for nn_AttentionLayer_41188736368660.

Reference math (B=16, S=8192, D_MODEL=K_CH=OUT=256):
    q   = query @ Wq + bq                       # [B, OUT]
    k   = key @ Wk + bk                         # [B, S, OUT]
    v   = value @ Wv + bv                       # [B, S, OUT]
    s   = (q . k_s) / sqrt(OUT)                 # [B, S]
    w   = softmax(s)                            # [B, S]
    ctx = w @ v                                 # [B, OUT]
    out = broadcast ctx over S                  # [B, S, OUT]

Algebraic restructuring (exact):
    q . (key_s @ Wk + bk) = key_s . (Wk @ q) + const   (const cancels in softmax)
    w @ (value @ Wv + bv) = (w @ value) @ Wv + bv      (sum w = 1)
so the device only streams key and value once:
    s_s = key_s . qk,  e = exp(s),  T = sum e,  u = (e @ value) / T
with qk = Wk @ (query @ Wq + bq) * scale computed on host (tiny).

Device architecture (fp8, PE-centric):
  * key and value are cast to fp8-e4m3 on the host (whole-pipeline rel err
    vs the fp32 reference is ~9e-3, well under the 2e-2 gate; the HBM
    stream -- the roofline limit -- shrinks 4x vs fp32).
  * key is stored CHANNEL-major ([c, s] per batch), so the score
    dot-products run on the PE array: lhsT = qk half [128c, 32 replicas],
    rhs = key tile [128c, 512 positions] -> scores [32 replicas, 512] in
    PSUM, accumulated over the two 128-channel halves.  4 such groups pack
    one PSUM bank at partition offsets 0/32/64/96 (legal tile_position),
    so one ACT exp instruction covers a whole bank (2048 positions).
  * the 4 banks' exps of a batch land in ONE SBUF tile; a single
    flatten-scatter DMA per batch (128 descriptors of 128B -- descriptor
    count is what the DGE ring pays for) redistributes them to [128, 64]:
    each partition gets the weights of its 64 value rows.  Value is
    permuted on the host to match the scatter's stream order.
  * value carries a 257th all-ones channel, so the same PE accumulation
    that produces u also produces T = sum(w) -- no separate row-sum
    reduction or [128,x] store.
  * weighted value sum runs on PE: M=1 matmuls accumulating into 4 PSUM
    column strips (concurrent array column-strips).
  * host does the final tiny merge, 1/T normalize, Wv projection,
    broadcast.

Sharding: data-parallel over batch, B=16 -> 2 batches per core x 8 cores,
no cross-core communication.
"""

import numpy as np
import ml_dtypes

import concourse.bass as bass
import concourse.tile as tile
from concourse import mybir
from concourse.bass_utils import run_bass_kernel_spmd

B, S, C = 16, 8192, 256
CP = C + 1           # value channels + ones column (T accumulator)
N_CORES = 8
BPC = B // N_CORES   # batches per core
P = 128
NB = 4               # score PSUM banks per batch (2048 positions each)
NG = 4               # score groups per bank (one per 32-partition block)
GN = 512             # positions per score matmul (one PSUM bank row)
NCH = 64             # value chunks per batch (128 positions each)
MREP = 32            # qk replication in the score lhsT
SCALE = 1.0 / 16.0
F32 = mybir.dt.float32
BF16 = mybir.dt.bfloat16
F8 = mybir.dt.float8e4

FP8NP = ml_dtypes.float8_e4m3
BF16NP = ml_dtypes.bfloat16

_NC = None

# position of value row (p, j):  s = 2048*bank + 512*a + 64*r + j
# with a = p//32, bank = (p%32)//8, r = p%8  -- this matches the stream
# order of the per-batch scatter DMA (see _build_nc).
_PIDX = np.arange(P)
_POS = (2048 * ((_PIDX % 32) // 8) + 512 * (_PIDX // 32) + 64 * (_PIDX % 8))[
    :, None
] + np.arange(NCH)[None, :]


def _build_nc(split_waits=True):
    nc = bass.Bass("TRN2", target_bir_lowering=False, debug=False)

    # [b, c-half, c_local, s] channel-major key
    keyt_d = nc.dram_tensor("keyt", [BPC, 2, P, S], F8, kind="ExternalInput")
    # [b, p, j, c] value permuted to scatter order, plus ones channel
    valp_d = nc.dram_tensor("valp", [BPC, P, NCH, CP], F8, kind="ExternalInput")
    # [c_local, b, c-half, replica]
    qkb_d = nc.dram_tensor("qkb", [P, BPC, 2, MREP], BF16, kind="ExternalInput")
    u_d = nc.dram_tensor("u", [BPC, 4 * CP], F32, kind="ExternalOutput")

    # key tiles [128c, 4096s]; value tiles [128p, 32*CP]
    keyt_v = keyt_d.ap().rearrange("b h c (sh s) -> b h sh c s", sh=2)
    valp_v = valp_d.ap().rearrange("b p (jh j) c -> b jh p (j c)", jh=2)
    qkb_v = qkb_d.ap().rearrange("p b h m -> p (b h m)")

    with tile.TileContext(nc) as tc:
        with (
            tc.tile_pool(name="kpool", bufs=8) as kpool,
            tc.tile_pool(name="vpool", bufs=4) as vpool,
            tc.tile_pool(name="wpool", bufs=2) as wpool,
            tc.tile_pool(name="wcpool", bufs=2) as wcpool,
            tc.tile_pool(name="cpool", bufs=1) as cpool,
            tc.tile_pool(name="spool", bufs=4, space="PSUM") as spool,
            tc.tile_pool(name="apool", bufs=2, space="PSUM") as apool,
        ):
            # First key tile before the tiny constant load: the big stream
            # (SP ring) starts immediately; small DMAs (qkb, scatters,
            # stores) ride the ACT ring and never stall the stream.
            kt00 = kpool.tile([P, S // 2], F8, tag="kt")
            nc.sync.dma_start(out=kt00[:], in_=keyt_v[0, 0, 0])

            qkb_t = cpool.tile([P, BPC * 2 * MREP], BF16, tag="qkb")
            nc.scalar.dma_start(out=qkb_t[:], in_=qkb_v)

            kts = {}
            vts = {}

            def load_k(b, ch, sh):
                if (b, ch, sh) == (0, 0, 0):
                    kts[(b, ch, sh)] = kt00
                    return
                t = kpool.tile([P, S // 2], F8, tag="kt")
                nc.sync.dma_start(out=t[:], in_=keyt_v[b, ch, sh])
                kts[(b, ch, sh)] = t

            def load_v(b, jh):
                t = vpool.tile([P, (NCH // 2) * CP], F8, tag="vt")
                nc.sync.dma_start(out=t[:], in_=valp_v[b, jh])
                vts[(b, jh)] = t

            # SP ring order: both batches' keys stream before the late
            # value tiles, so all score banks + exp/scatter chains finish
            # mid-stream and each value block starts right as its tile
            # lands.  The last value tile's matmuls are the only
            # post-stream work.
            load_k(0, 0, 0); load_k(0, 1, 0); load_k(0, 0, 1); load_k(0, 1, 1)
            load_v(0, 0); load_v(0, 1)
            load_k(1, 0, 0); load_k(1, 1, 0); load_k(1, 0, 1); load_k(1, 1, 1)
            load_v(1, 0); load_v(1, 1)

            wx_all = {}
            wcol_all = {}
            u_ps = {}
            u4 = {}

            def scores_bank(b, bank):
                """8 matmuls -> one exp into the batch's wx tile."""
                if bank == 0:
                    wx_t = wpool.tile([P, NB * GN], BF16, tag="wx")
                    wx_all[b] = wx_t
                h2, q = bank // 2, bank % 2
                ps = spool.tile([P, GN], F32, tag="ps")
                for ch in range(2):
                    lhsT = qkb_t[:, (b * 2 + ch) * MREP:(b * 2 + ch + 1) * MREP]
                    kt = kts[(b, ch, h2)]
                    for g in range(NG):
                        lo = q * (NG * GN) + g * GN
                        nc.tensor.matmul(
                            out=ps[g * MREP:(g + 1) * MREP, :],
                            lhsT=lhsT,
                            rhs=kt[:, lo:lo + GN],
                            start=(ch == 0),
                            stop=(ch == 1),
                            tile_position=(0, g * MREP),
                            skip_group_check=True,
                        )
                nc.scalar.activation(
                    out=wx_all[b][:, bank * GN:(bank + 1) * GN], in_=ps[:],
                    func=mybir.ActivationFunctionType.Exp,
                )

            def scatter_batch(b):
                """One DMA: wx rows {0,32,64,96} x (bank, n) -> [128, 64]."""
                wxa = wx_all[b][:]
                src = type(wxa)(
                    tensor=wxa.tensor,
                    offset=wxa.offset,
                    ap=[[MREP * NB * GN, NG], [GN, NB], [1, GN]],
                )
                wcol = wcpool.tile([P, NCH], BF16, tag="wcol")
                nc.scalar.dma_start(out=wcol[:], in_=src)
                wcol_all[b] = wcol

            def values_half(b, jh):
                wcol = wcol_all[b]
                vt = vts[(b, jh)]
                for jj in range(NCH // 2):
                    j = jh * (NCH // 2) + jj
                    g4 = j % 4
                    nc.tensor.matmul(
                        out=u_ps[b][g4 * 32:g4 * 32 + 1, :],
                        lhsT=wcol[:, j:j + 1],
                        rhs=vt[:, jj * CP:(jj + 1) * CP],
                        start=(j < 4),
                        stop=(j >= NCH - 4),
                        tile_position=(0, g4 * 32),
                        skip_group_check=True,
                    )

            def finish_batch(b):
                # strip copies PSUM->SBUF split DVE/ACT
                u4_t = cpool.tile([1, 4 * CP], F32, tag=f"u4{b}")
                u4[b] = u4_t
                for g in range(4):
                    dst = u4_t[:, g * CP:(g + 1) * CP]
                    srcp = u_ps[b][g * 32:g * 32 + 1, :]
                    if g % 2 == 0:
                        nc.vector.tensor_copy(dst, srcp)
                    else:
                        nc.scalar.activation(
                            out=dst, in_=srcp,
                            func=mybir.ActivationFunctionType.Copy,
                        )

            for b in range(BPC):
                ups_t = apool.tile([P, CP], F32, tag=f"ups{b}")
                u_ps[b] = ups_t

            # PE emission order is monotone with data arrival: each batch's
            # scores, then its value blocks; the PSUM->SBUF strip copies
            # and stores form one short uniform tail at the end.
            for bank in range(NB):
                scores_bank(0, bank)
            scatter_batch(0)
            values_half(0, 0)
            values_half(0, 1)
            for bank in range(NB):
                scores_bank(1, bank)
            scatter_batch(1)
            values_half(1, 0)
            values_half(1, 1)
            finish_batch(0)
            finish_batch(1)

            for b in range(BPC):
                nc.scalar.dma_start(out=u_d.ap()[b:b + 1, :], in_=u4[b][:])

    if split_waits:
        _split_multi_waits(nc)
    return nc


def _split_multi_waits(nc, max_waits=1):
    """Walrus encodes at most one sync-wait per TPB instruction ("Too many
    sync wait commands").  Hoist extra waits onto standalone EventSemaphore
    instructions inserted immediately before, on the same engine stream --
    semantically identical, no reordering."""
    n_split = 0
    for f in nc.m.functions:
        for blk in f.blocks:
            il = blk.instructions
            i = 0
            while i < len(il):
                inst = il[i]
                si = inst.sync_info
                if si is not None and len(si.on_wait) > max_waits:
                    waits = list(si.on_wait)
                    extra, keep = waits[:-max_waits], waits[-max_waits:]
                    for k, w in enumerate(extra):
                        ev = mybir.InstEventSemaphore(
                            name=f"{inst.name}-wsplit{k}",
                            engine=inst.engine,
                            ins=[],
                            outs=[],
                            sync_info=mybir.SyncInfo(on_wait=[w], on_update=[]),
                        )
                        il.insert(i, ev)
                        i += 1
                        n_split += 1
                    inst.sync_info = mybir.SyncInfo(
                        on_wait=keep, on_update=list(si.on_update)
                    )
                i += 1
    return n_split


def get_nc():
    global _NC
    if _NC is None:
        _NC = _build_nc()
    return _NC


def host_pre(query, Wq, bq, Wk):
    q = query @ Wq + bq          # [B, OUT]
    qk = q @ Wk.T                # [B, K_CH]
    return (qk * SCALE).astype(np.float32)


def make_in_maps(key, value, qk):
    """Per-core input maps for run_bass_kernel_spmd."""
    # qkb: [c_local, b, c-half, replica] bf16
    qkh = qk.reshape(B, 2, P).transpose(2, 0, 1)            # [128, B, 2]
    qkb = np.ascontiguousarray(
        np.broadcast_to(qkh[:, :, :, None], (P, B, 2, MREP))
    ).astype(BF16NP)
    in_maps = []
    for c in range(N_CORES):
        sl = slice(c * BPC, (c + 1) * BPC)
        keyt = np.ascontiguousarray(
            key[sl].transpose(0, 2, 1)
        ).reshape(BPC, 2, P, S).astype(FP8NP)
        # value rows permuted into scatter-stream order + ones channel
        vperm = value[sl][:, _POS.reshape(-1), :].reshape(BPC, P, NCH, C)
        valp = np.empty((BPC, P, NCH, CP), dtype=FP8NP)
        valp[..., :C] = vperm.astype(FP8NP)
        valp[..., C] = FP8NP(1.0)
        in_maps.append(
            {
                "keyt": keyt,
                "valp": valp,
                "qkb": np.ascontiguousarray(qkb[:, sl]),
            }
        )
    return in_maps


def host_post(u, Wv, bv):
    ctx = (u @ Wv + bv).astype(np.float32)   # [B, OUT]
    return np.broadcast_to(ctx[:, None, :], (B, S, C))


def kernel(query, key, value, Wq, bq, Wk, bk, Wv, bv, _results=None, _run_kwargs=None):
    query = np.asarray(query, np.float32)
    key = np.asarray(key, np.float32)
    value = np.asarray(value, np.float32)
    Wq = np.asarray(Wq, np.float32)
    bq = np.asarray(bq, np.float32)
    Wk = np.asarray(Wk, np.float32)
    Wv = np.asarray(Wv, np.float32)
    bv = np.asarray(bv, np.float32)

    qk = host_pre(query, Wq, bq, Wk)
    nc = get_nc()
    in_maps = make_in_maps(key, value, qk)
    res = run_bass_kernel_spmd(
        nc, in_maps, list(range(N_CORES)), **(_run_kwargs or {})
    )
    if _results is not None:
        _results.append(res)
    us = []
    for c in range(N_CORES):
        u4 = res.results[c]["u"].reshape(BPC, 4, CP).sum(axis=1)
        us.append(u4[:, :C] / u4[:, C:])
    u = np.concatenate(us, axis=0)
    return host_post(u, Wv, bv)


# revision 21
# speedup vs baseline: 1.1501x; 1.1501x over previous
"""Trainium2 Bass kernel for nn_AttentionLayer_41188736368660.

Reference math (B=16, S=8192, D_MODEL=K_CH=OUT=256):
    q   = query @ Wq + bq                       # [B, OUT]
    k   = key @ Wk + bk                         # [B, S, OUT]
    v   = value @ Wv + bv                       # [B, S, OUT]
    s   = (q . k_s) / sqrt(OUT)                 # [B, S]
    w   = softmax(s)                            # [B, S]
    ctx = w @ v                                 # [B, OUT]
    out = broadcast ctx over S                  # [B, S, OUT]

Algebraic restructuring (exact):
    q . (key_s @ Wk + bk) = key_s . (Wk @ q) + const   (const cancels in softmax)
    w @ (value @ Wv + bv) = (w @ value) @ Wv + bv      (sum w = 1)
so the device only streams key and value once:
    s_s = key_s . qk,  e = exp(s),  T = sum e,  u = (e @ value) / T
with qk = Wk @ (query @ Wq + bq) * scale computed on host (tiny).

Device architecture (fp8, PE-centric):
  * key and value are cast to fp8-e4m3 on the host (whole-pipeline rel err
    vs the fp32 reference is ~9e-3, well under the 2e-2 gate; the HBM
    stream -- the roofline limit -- shrinks 4x vs fp32).
  * key is stored CHANNEL-major ([c, s] per batch), so the score
    dot-products run on the PE array: lhsT = qk half [128c, 32 replicas],
    rhs = key tile [128c, 512 positions] -> scores [32 replicas, 512] in
    PSUM, accumulated over the two 128-channel halves.  4 such groups pack
    one PSUM bank at partition offsets 0/32/64/96 (legal tile_position),
    so one ACT exp instruction covers a whole bank (2048 positions).
  * the 4 banks' exps of a batch land in ONE SBUF tile; a single
    flatten-scatter DMA per batch (128 descriptors of 128B -- descriptor
    count is what the DGE ring pays for) redistributes them to [128, 64]:
    each partition gets the weights of its 64 value rows.  Value is
    permuted on the host to match the scatter's stream order.
  * value carries a 257th all-ones channel, so the same PE accumulation
    that produces u also produces T = sum(w) -- no separate row-sum
    reduction or [128,x] store.
  * weighted value sum runs on PE: M=1 matmuls accumulating into 4 PSUM
    column strips (concurrent array column-strips).
  * host does the final tiny merge, 1/T normalize, Wv projection,
    broadcast.

Sharding: data-parallel over batch, B=16 -> 2 batches per core x 8 cores,
no cross-core communication.
"""

import numpy as np
import ml_dtypes

import concourse.bass as bass
import concourse.tile as tile
from concourse import mybir
from concourse.bass_utils import run_bass_kernel_spmd

B, S, C = 16, 8192, 256
CP = C + 1           # value channels + ones column (T accumulator)
N_CORES = 8
BPC = B // N_CORES   # batches per core
P = 128
NB = 4               # score PSUM banks per batch (2048 positions each)
NG = 4               # score groups per bank (one per 32-partition block)
GN = 512             # positions per score matmul (one PSUM bank row)
NCH = 64             # value chunks per batch (128 positions each)
MREP = 32            # qk replication in the score lhsT
SCALE = 1.0 / 16.0
F32 = mybir.dt.float32
BF16 = mybir.dt.bfloat16
F8 = mybir.dt.float8e4

FP8NP = ml_dtypes.float8_e4m3
BF16NP = ml_dtypes.bfloat16

_NC = None

# position of value row (p, j):  s = 2048*bank + 512*a + 64*r + j
# with a = p//32, bank = (p%32)//8, r = p%8  -- this matches the stream
# order of the per-batch scatter DMA (see _build_nc).
_PIDX = np.arange(P)
_POS = (2048 * ((_PIDX % 32) // 8) + 512 * (_PIDX // 32) + 64 * (_PIDX % 8))[
    :, None
] + np.arange(NCH)[None, :]


def _build_nc(split_waits=True):
    nc = bass.Bass("TRN2", target_bir_lowering=False, debug=False)

    # [b, c-half, c_local, s] channel-major key
    keyt_d = nc.dram_tensor("keyt", [BPC, 2, P, S], F8, kind="ExternalInput")
    # [b, p, j, c] value permuted to scatter order, plus ones channel
    valp_d = nc.dram_tensor("valp", [BPC, P, NCH, CP], F8, kind="ExternalInput")
    # [c_local, b, c-half, replica]
    qkb_d = nc.dram_tensor("qkb", [P, BPC, 2, MREP], BF16, kind="ExternalInput")
    u_d = nc.dram_tensor("u", [BPC, 4 * CP], F32, kind="ExternalOutput")

    # key tiles [128c, 4096s]; value tiles [128p, 32*CP]
    keyt_v = keyt_d.ap().rearrange("b h c (sh s) -> b h sh c s", sh=2)
    valp_v = valp_d.ap().rearrange("b p (jh j) c -> b jh p (j c)", jh=2)
    qkb_v = qkb_d.ap().rearrange("p b h m -> p (b h m)")

    with tile.TileContext(nc) as tc:
        with (
            tc.tile_pool(name="kpool", bufs=8) as kpool,
            tc.tile_pool(name="vpool", bufs=4) as vpool,
            tc.tile_pool(name="wpool", bufs=2) as wpool,
            tc.tile_pool(name="wcpool", bufs=2) as wcpool,
            tc.tile_pool(name="cpool", bufs=1) as cpool,
            tc.tile_pool(name="spool", bufs=4, space="PSUM") as spool,
            tc.tile_pool(name="apool", bufs=2, space="PSUM") as apool,
        ):
            # First key tile before the tiny constant load: the big stream
            # (SP ring) starts immediately; small DMAs (qkb, scatters,
            # stores) ride the ACT ring and never stall the stream.
            kt00 = kpool.tile([P, S // 2], F8, tag="kt")
            nc.sync.dma_start(out=kt00[:], in_=keyt_v[0, 0, 0])

            qkb_t = cpool.tile([P, BPC * 2 * MREP], BF16, tag="qkb")
            nc.scalar.dma_start(out=qkb_t[:], in_=qkb_v)

            kts = {}
            vts = {}

            def load_k(b, ch, sh):
                if (b, ch, sh) == (0, 0, 0):
                    kts[(b, ch, sh)] = kt00
                    return
                t = kpool.tile([P, S // 2], F8, tag="kt")
                nc.sync.dma_start(out=t[:], in_=keyt_v[b, ch, sh])
                kts[(b, ch, sh)] = t

            def load_v(b, jh):
                t = vpool.tile([P, (NCH // 2) * CP], F8, tag="vt")
                nc.sync.dma_start(out=t[:], in_=valp_v[b, jh])
                vts[(b, jh)] = t

            # SP ring order: both batches' keys stream before the late
            # value tiles, so all score banks + exp/scatter chains finish
            # mid-stream and each value block starts right as its tile
            # lands.  The last value tile's matmuls are the only
            # post-stream work.
            load_k(0, 0, 0); load_k(0, 1, 0); load_k(0, 0, 1); load_k(0, 1, 1)
            load_v(0, 0)
            load_k(1, 0, 0); load_k(1, 1, 0); load_k(1, 0, 1); load_k(1, 1, 1)
            load_v(0, 1)
            load_v(1, 0); load_v(1, 1)

            wx_all = {}
            wcol_all = {}
            u_ps = {}
            u4 = {}

            def scores_bank(b, bank):
                """8 matmuls -> one exp into the batch's wx tile."""
                if bank == 0:
                    wx_t = wpool.tile([P, NB * GN], BF16, tag="wx")
                    wx_all[b] = wx_t
                h2, q = bank // 2, bank % 2
                ps = spool.tile([P, GN], F32, tag="ps")
                for ch in range(2):
                    lhsT = qkb_t[:, (b * 2 + ch) * MREP:(b * 2 + ch + 1) * MREP]
                    kt = kts[(b, ch, h2)]
                    for g in range(NG):
                        lo = q * (NG * GN) + g * GN
                        nc.tensor.matmul(
                            out=ps[g * MREP:(g + 1) * MREP, :],
                            lhsT=lhsT,
                            rhs=kt[:, lo:lo + GN],
                            start=(ch == 0),
                            stop=(ch == 1),
                            tile_position=(0, g * MREP),
                            skip_group_check=True,
                        )
                nc.scalar.activation(
                    out=wx_all[b][:, bank * GN:(bank + 1) * GN], in_=ps[:],
                    func=mybir.ActivationFunctionType.Exp,
                )

            def scatter_batch(b):
                """One DMA: wx rows {0,32,64,96} x (bank, n) -> [128, 64]."""
                wxa = wx_all[b][:]
                src = type(wxa)(
                    tensor=wxa.tensor,
                    offset=wxa.offset,
                    ap=[[MREP * NB * GN, NG], [GN, NB], [1, GN]],
                )
                wcol = wcpool.tile([P, NCH], BF16, tag="wcol")
                nc.scalar.dma_start(out=wcol[:], in_=src)
                wcol_all[b] = wcol

            def values_half(b, jh):
                wcol = wcol_all[b]
                vt = vts[(b, jh)]
                for jj in range(NCH // 2):
                    j = jh * (NCH // 2) + jj
                    g4 = j % 4
                    nc.tensor.matmul(
                        out=u_ps[b][g4 * 32:g4 * 32 + 1, :],
                        lhsT=wcol[:, j:j + 1],
                        rhs=vt[:, jj * CP:(jj + 1) * CP],
                        start=(j < 4),
                        stop=(j >= NCH - 4),
                        tile_position=(0, g4 * 32),
                        skip_group_check=True,
                    )

            def finish_batch(b):
                # strip copies PSUM->SBUF split DVE/ACT
                u4_t = cpool.tile([1, 4 * CP], F32, tag=f"u4{b}")
                u4[b] = u4_t
                for g in range(4):
                    dst = u4_t[:, g * CP:(g + 1) * CP]
                    srcp = u_ps[b][g * 32:g * 32 + 1, :]
                    if g % 2 == 0:
                        nc.vector.tensor_copy(dst, srcp)
                    else:
                        nc.scalar.activation(
                            out=dst, in_=srcp,
                            func=mybir.ActivationFunctionType.Copy,
                        )

            for b in range(BPC):
                ups_t = apool.tile([P, CP], F32, tag=f"ups{b}")
                u_ps[b] = ups_t

            # PE emission order is monotone with data arrival: each batch's
            # scores, then its value blocks; the PSUM->SBUF strip copies
            # and stores form one short uniform tail at the end.
            for bank in range(NB):
                scores_bank(0, bank)
            scatter_batch(0)
            for bank in range(NB):
                scores_bank(1, bank)
            scatter_batch(1)
            values_half(0, 0)
            values_half(0, 1)
            values_half(1, 0)
            values_half(1, 1)
            finish_batch(0)
            finish_batch(1)

            for b in range(BPC):
                nc.scalar.dma_start(out=u_d.ap()[b:b + 1, :], in_=u4[b][:])

    if split_waits:
        _split_multi_waits(nc)
    return nc


def _split_multi_waits(nc, max_waits=1):
    """Walrus encodes at most one sync-wait per TPB instruction ("Too many
    sync wait commands").  Hoist extra waits onto standalone EventSemaphore
    instructions inserted immediately before, on the same engine stream --
    semantically identical, no reordering."""
    n_split = 0
    for f in nc.m.functions:
        for blk in f.blocks:
            il = blk.instructions
            i = 0
            while i < len(il):
                inst = il[i]
                si = inst.sync_info
                if si is not None and len(si.on_wait) > max_waits:
                    waits = list(si.on_wait)
                    extra, keep = waits[:-max_waits], waits[-max_waits:]
                    for k, w in enumerate(extra):
                        ev = mybir.InstEventSemaphore(
                            name=f"{inst.name}-wsplit{k}",
                            engine=inst.engine,
                            ins=[],
                            outs=[],
                            sync_info=mybir.SyncInfo(on_wait=[w], on_update=[]),
                        )
                        il.insert(i, ev)
                        i += 1
                        n_split += 1
                    inst.sync_info = mybir.SyncInfo(
                        on_wait=keep, on_update=list(si.on_update)
                    )
                i += 1
    return n_split


def get_nc():
    global _NC
    if _NC is None:
        _NC = _build_nc()
    return _NC


def host_pre(query, Wq, bq, Wk):
    q = query @ Wq + bq          # [B, OUT]
    qk = q @ Wk.T                # [B, K_CH]
    return (qk * SCALE).astype(np.float32)


def make_in_maps(key, value, qk):
    """Per-core input maps for run_bass_kernel_spmd."""
    # qkb: [c_local, b, c-half, replica] bf16
    qkh = qk.reshape(B, 2, P).transpose(2, 0, 1)            # [128, B, 2]
    qkb = np.ascontiguousarray(
        np.broadcast_to(qkh[:, :, :, None], (P, B, 2, MREP))
    ).astype(BF16NP)
    in_maps = []
    for c in range(N_CORES):
        sl = slice(c * BPC, (c + 1) * BPC)
        keyt = np.ascontiguousarray(
            key[sl].transpose(0, 2, 1)
        ).reshape(BPC, 2, P, S).astype(FP8NP)
        # value rows permuted into scatter-stream order + ones channel
        vperm = value[sl][:, _POS.reshape(-1), :].reshape(BPC, P, NCH, C)
        valp = np.empty((BPC, P, NCH, CP), dtype=FP8NP)
        valp[..., :C] = vperm.astype(FP8NP)
        valp[..., C] = FP8NP(1.0)
        in_maps.append(
            {
                "keyt": keyt,
                "valp": valp,
                "qkb": np.ascontiguousarray(qkb[:, sl]),
            }
        )
    return in_maps


def host_post(u, Wv, bv):
    ctx = (u @ Wv + bv).astype(np.float32)   # [B, OUT]
    return np.broadcast_to(ctx[:, None, :], (B, S, C))


def kernel(query, key, value, Wq, bq, Wk, bk, Wv, bv, _results=None, _run_kwargs=None):
    query = np.asarray(query, np.float32)
    key = np.asarray(key, np.float32)
    value = np.asarray(value, np.float32)
    Wq = np.asarray(Wq, np.float32)
    bq = np.asarray(bq, np.float32)
    Wk = np.asarray(Wk, np.float32)
    Wv = np.asarray(Wv, np.float32)
    bv = np.asarray(bv, np.float32)

    qk = host_pre(query, Wq, bq, Wk)
    nc = get_nc()
    in_maps = make_in_maps(key, value, qk)
    res = run_bass_kernel_spmd(
        nc, in_maps, list(range(N_CORES)), **(_run_kwargs or {})
    )
    if _results is not None:
        _results.append(res)
    us = []
    for c in range(N_CORES):
        u4 = res.results[c]["u"].reshape(BPC, 4, CP).sum(axis=1)
        us.append(u4[:, :C] / u4[:, C:])
    u = np.concatenate(us, axis=0)
    return host_post(u, Wv, bv)


# revision 22
# speedup vs baseline: 1.1884x; 1.0333x over previous
"""Trainium2 Bass kernel for nn_AttentionLayer_41188736368660.

Reference math (B=16, S=8192, D_MODEL=K_CH=OUT=256):
    q   = query @ Wq + bq                       # [B, OUT]
    k   = key @ Wk + bk                         # [B, S, OUT]
    v   = value @ Wv + bv                       # [B, S, OUT]
    s   = (q . k_s) / sqrt(OUT)                 # [B, S]
    w   = softmax(s)                            # [B, S]
    ctx = w @ v                                 # [B, OUT]
    out = broadcast ctx over S                  # [B, S, OUT]

Algebraic restructuring (exact):
    q . (key_s @ Wk + bk) = key_s . (Wk @ q) + const   (const cancels in softmax)
    w @ (value @ Wv + bv) = (w @ value) @ Wv + bv      (sum w = 1)
so the device only streams key and value once:
    s_s = key_s . qk,  e = exp(s),  T = sum e,  u = (e @ value) / T
with qk = Wk @ (query @ Wq + bq) * scale computed on host (tiny).

Device architecture (fp8, PE-centric):
  * key and value are cast to fp8-e4m3 on the host (whole-pipeline rel err
    vs the fp32 reference is ~9e-3, well under the 2e-2 gate; the HBM
    stream -- the roofline limit -- shrinks 4x vs fp32).
  * key is stored CHANNEL-major ([c, s] per batch), so the score
    dot-products run on the PE array: lhsT = qk half [128c, 32 replicas],
    rhs = key tile [128c, 512 positions] -> scores [32 replicas, 512] in
    PSUM, accumulated over the two 128-channel halves.  4 such groups pack
    one PSUM bank at partition offsets 0/32/64/96 (legal tile_position),
    so one ACT exp instruction covers a whole bank (2048 positions).
  * the 4 banks' exps of a batch land in ONE SBUF tile; a single
    flatten-scatter DMA per batch (128 descriptors of 128B -- descriptor
    count is what the DGE ring pays for) redistributes them to [128, 64]:
    each partition gets the weights of its 64 value rows.  Value is
    permuted on the host to match the scatter's stream order.
  * value carries a 257th all-ones channel, so the same PE accumulation
    that produces u also produces T = sum(w) -- no separate row-sum
    reduction or [128,x] store.
  * weighted value sum runs on PE: M=1 matmuls accumulating into 4 PSUM
    column strips (concurrent array column-strips).
  * host does the final tiny merge, 1/T normalize, Wv projection,
    broadcast.

Sharding: data-parallel over batch, B=16 -> 2 batches per core x 8 cores,
no cross-core communication.
"""

import numpy as np
import ml_dtypes

import concourse.bass as bass
import concourse.tile as tile
from concourse import mybir
from concourse.bass_utils import run_bass_kernel_spmd

B, S, C = 16, 8192, 256
CP = C + 1           # value channels + ones column (T accumulator)
N_CORES = 8
BPC = B // N_CORES   # batches per core
P = 128
NB = 4               # score PSUM banks per batch (2048 positions each)
NG = 4               # score groups per bank (one per 32-partition block)
GN = 512             # positions per score matmul (one PSUM bank row)
NCH = 64             # value chunks per batch (128 positions each)
MREP = 32            # qk replication in the score lhsT
SCALE = 1.0 / 16.0
F32 = mybir.dt.float32
BF16 = mybir.dt.bfloat16
F8 = mybir.dt.float8e4

FP8NP = ml_dtypes.float8_e4m3
BF16NP = ml_dtypes.bfloat16

_NC = None

# position of value row (p, j):  s = 2048*bank + 512*a + 64*r + j
# with a = p//32, bank = (p%32)//8, r = p%8  -- this matches the stream
# order of the per-batch scatter DMA (see _build_nc).
_PIDX = np.arange(P)
_POS = (2048 * ((_PIDX % 32) // 8) + 512 * (_PIDX // 32) + 64 * (_PIDX % 8))[
    :, None
] + np.arange(NCH)[None, :]


def _build_nc(split_waits=True):
    nc = bass.Bass("TRN2", target_bir_lowering=False, debug=False)

    # [b, c-half, c_local, s] channel-major key
    keyt_d = nc.dram_tensor("keyt", [BPC, 2, P, S], F8, kind="ExternalInput")
    # [b, p, j, c] value permuted to scatter order, plus ones channel
    valp_d = nc.dram_tensor("valp", [BPC, P, NCH, CP], F8, kind="ExternalInput")
    # [c_local, b, c-half, replica]
    qkb_d = nc.dram_tensor("qkb", [P, BPC, 2, MREP], BF16, kind="ExternalInput")
    u_d = nc.dram_tensor("u", [BPC, 4 * CP], F32, kind="ExternalOutput")

    # key tiles [128c, 4096s]; value tiles [128p, 32*CP]
    keyt_v = keyt_d.ap().rearrange("b h c (sh s) -> b h sh c s", sh=2)
    valp_v = valp_d.ap().rearrange("b p (jq j) c -> b jq p (j c)", jq=4)
    qkb_v = qkb_d.ap().rearrange("p b h m -> p (b h m)")

    with tile.TileContext(nc) as tc:
        with (
            tc.tile_pool(name="kpool", bufs=8) as kpool,
            tc.tile_pool(name="vpool", bufs=8) as vpool,
            tc.tile_pool(name="wpool", bufs=2) as wpool,
            tc.tile_pool(name="wcpool", bufs=2) as wcpool,
            tc.tile_pool(name="cpool", bufs=1) as cpool,
            tc.tile_pool(name="spool", bufs=4, space="PSUM") as spool,
            tc.tile_pool(name="apool", bufs=2, space="PSUM") as apool,
        ):
            # First key tile before the tiny constant load: the big stream
            # (SP ring) starts immediately; small DMAs (qkb, scatters,
            # stores) ride the ACT ring and never stall the stream.
            kt00 = kpool.tile([P, S // 2], F8, tag="kt")
            nc.sync.dma_start(out=kt00[:], in_=keyt_v[0, 0, 0])

            qkb_t = cpool.tile([P, BPC * 2 * MREP], BF16, tag="qkb")
            nc.scalar.dma_start(out=qkb_t[:], in_=qkb_v)

            kts = {}
            vts = {}

            def load_k(b, ch, sh):
                if (b, ch, sh) == (0, 0, 0):
                    kts[(b, ch, sh)] = kt00
                    return
                t = kpool.tile([P, S // 2], F8, tag="kt")
                nc.sync.dma_start(out=t[:], in_=keyt_v[b, ch, sh])
                kts[(b, ch, sh)] = t

            def load_v(b, jq):
                t = vpool.tile([P, (NCH // 4) * CP], F8, tag="vt")
                nc.sync.dma_start(out=t[:], in_=valp_v[b, jq])
                vts[(b, jq)] = t

            # SP ring order: both batches' keys stream before the late
            # value tiles, so all score banks + exp/scatter chains finish
            # mid-stream and each value block starts right as its tile
            # lands.  The last value tile's matmuls are the only
            # post-stream work.
            load_k(0, 0, 0); load_k(0, 1, 0); load_k(0, 0, 1); load_k(0, 1, 1)
            load_k(1, 0, 0); load_k(1, 1, 0); load_k(1, 0, 1); load_k(1, 1, 1)
            for b in range(BPC):
                for jq in range(4):
                    load_v(b, jq)

            wx_all = {}
            wcol_all = {}
            u_ps = {}
            u4 = {}

            def scores_bank(b, bank):
                """8 matmuls -> one exp into the batch's wx tile."""
                if bank == 0:
                    wx_t = wpool.tile([P, NB * GN], BF16, tag="wx")
                    wx_all[b] = wx_t
                h2, q = bank // 2, bank % 2
                ps = spool.tile([P, GN], F32, tag="ps")
                for ch in range(2):
                    lhsT = qkb_t[:, (b * 2 + ch) * MREP:(b * 2 + ch + 1) * MREP]
                    kt = kts[(b, ch, h2)]
                    for g in range(NG):
                        lo = q * (NG * GN) + g * GN
                        nc.tensor.matmul(
                            out=ps[g * MREP:(g + 1) * MREP, :],
                            lhsT=lhsT,
                            rhs=kt[:, lo:lo + GN],
                            start=(ch == 0),
                            stop=(ch == 1),
                            tile_position=(0, g * MREP),
                            skip_group_check=True,
                        )
                nc.scalar.activation(
                    out=wx_all[b][:, bank * GN:(bank + 1) * GN], in_=ps[:],
                    func=mybir.ActivationFunctionType.Exp,
                )

            def scatter_batch(b):
                """One DMA: wx rows {0,32,64,96} x (bank, n) -> [128, 64]."""
                wxa = wx_all[b][:]
                src = type(wxa)(
                    tensor=wxa.tensor,
                    offset=wxa.offset,
                    ap=[[MREP * NB * GN, NG], [GN, NB], [1, GN]],
                )
                wcol = wcpool.tile([P, NCH], BF16, tag="wcol")
                nc.scalar.dma_start(out=wcol[:], in_=src)
                wcol_all[b] = wcol

            def values_quarter(b, jq):
                wcol = wcol_all[b]
                vt = vts[(b, jq)]
                for jj in range(NCH // 4):
                    j = jq * (NCH // 4) + jj
                    g4 = j % 4
                    nc.tensor.matmul(
                        out=u_ps[b][g4 * 32:g4 * 32 + 1, :],
                        lhsT=wcol[:, j:j + 1],
                        rhs=vt[:, jj * CP:(jj + 1) * CP],
                        start=(j < 4),
                        stop=(j >= NCH - 4),
                        tile_position=(0, g4 * 32),
                        skip_group_check=True,
                    )

            def finish_batch(b):
                # strip copies PSUM->SBUF split DVE/ACT
                u4_t = cpool.tile([1, 4 * CP], F32, tag=f"u4{b}")
                u4[b] = u4_t
                for g in range(4):
                    dst = u4_t[:, g * CP:(g + 1) * CP]
                    srcp = u_ps[b][g * 32:g * 32 + 1, :]
                    if g % 2 == 0:
                        nc.vector.tensor_copy(dst, srcp)
                    else:
                        nc.scalar.activation(
                            out=dst, in_=srcp,
                            func=mybir.ActivationFunctionType.Copy,
                        )
                nc.scalar.dma_start(out=u_d.ap()[b:b + 1, :], in_=u4_t[:])

            for b in range(BPC):
                ups_t = apool.tile([P, CP], F32, tag=f"ups{b}")
                u_ps[b] = ups_t

            # PE emission order is monotone with data arrival: each batch's
            # scores, then its value blocks; the PSUM->SBUF strip copies
            # and stores form one short uniform tail at the end.
            for bank in range(NB):
                scores_bank(0, bank)
            scatter_batch(0)
            for bank in range(NB):
                scores_bank(1, bank)
            scatter_batch(1)
            for jq in range(4):
                values_quarter(0, jq)
            finish_batch(0)
            for jq in range(4):
                values_quarter(1, jq)
            finish_batch(1)



    if split_waits:
        _split_multi_waits(nc)
    return nc


def _split_multi_waits(nc, max_waits=1):
    """Walrus encodes at most one sync-wait per TPB instruction ("Too many
    sync wait commands").  Hoist extra waits onto standalone EventSemaphore
    instructions inserted immediately before, on the same engine stream --
    semantically identical, no reordering."""
    n_split = 0
    for f in nc.m.functions:
        for blk in f.blocks:
            il = blk.instructions
            i = 0
            while i < len(il):
                inst = il[i]
                si = inst.sync_info
                if si is not None and len(si.on_wait) > max_waits:
                    waits = list(si.on_wait)
                    extra, keep = waits[:-max_waits], waits[-max_waits:]
                    for k, w in enumerate(extra):
                        ev = mybir.InstEventSemaphore(
                            name=f"{inst.name}-wsplit{k}",
                            engine=inst.engine,
                            ins=[],
                            outs=[],
                            sync_info=mybir.SyncInfo(on_wait=[w], on_update=[]),
                        )
                        il.insert(i, ev)
                        i += 1
                        n_split += 1
                    inst.sync_info = mybir.SyncInfo(
                        on_wait=keep, on_update=list(si.on_update)
                    )
                i += 1
    return n_split


def get_nc():
    global _NC
    if _NC is None:
        _NC = _build_nc()
    return _NC


def host_pre(query, Wq, bq, Wk):
    q = query @ Wq + bq          # [B, OUT]
    qk = q @ Wk.T                # [B, K_CH]
    return (qk * SCALE).astype(np.float32)


def make_in_maps(key, value, qk):
    """Per-core input maps for run_bass_kernel_spmd."""
    # qkb: [c_local, b, c-half, replica] bf16
    qkh = qk.reshape(B, 2, P).transpose(2, 0, 1)            # [128, B, 2]
    qkb = np.ascontiguousarray(
        np.broadcast_to(qkh[:, :, :, None], (P, B, 2, MREP))
    ).astype(BF16NP)
    in_maps = []
    for c in range(N_CORES):
        sl = slice(c * BPC, (c + 1) * BPC)
        keyt = np.ascontiguousarray(
            key[sl].transpose(0, 2, 1)
        ).reshape(BPC, 2, P, S).astype(FP8NP)
        # value rows permuted into scatter-stream order + ones channel
        vperm = value[sl][:, _POS.reshape(-1), :].reshape(BPC, P, NCH, C)
        valp = np.empty((BPC, P, NCH, CP), dtype=FP8NP)
        valp[..., :C] = vperm.astype(FP8NP)
        valp[..., C] = FP8NP(1.0)
        in_maps.append(
            {
                "keyt": keyt,
                "valp": valp,
                "qkb": np.ascontiguousarray(qkb[:, sl]),
            }
        )
    return in_maps


def host_post(u, Wv, bv):
    ctx = (u @ Wv + bv).astype(np.float32)   # [B, OUT]
    return np.broadcast_to(ctx[:, None, :], (B, S, C))


def kernel(query, key, value, Wq, bq, Wk, bk, Wv, bv, _results=None, _run_kwargs=None):
    query = np.asarray(query, np.float32)
    key = np.asarray(key, np.float32)
    value = np.asarray(value, np.float32)
    Wq = np.asarray(Wq, np.float32)
    bq = np.asarray(bq, np.float32)
    Wk = np.asarray(Wk, np.float32)
    Wv = np.asarray(Wv, np.float32)
    bv = np.asarray(bv, np.float32)

    qk = host_pre(query, Wq, bq, Wk)
    nc = get_nc()
    in_maps = make_in_maps(key, value, qk)
    res = run_bass_kernel_spmd(
        nc, in_maps, list(range(N_CORES)), **(_run_kwargs or {})
    )
    if _results is not None:
        _results.append(res)
    us = []
    for c in range(N_CORES):
        u4 = res.results[c]["u"].reshape(BPC, 4, CP).sum(axis=1)
        us.append(u4[:, :C] / u4[:, C:])
    u = np.concatenate(us, axis=0)
    return host_post(u, Wv, bv)


# revision 23
# speedup vs baseline: 1.2266x; 1.0321x over previous
"""Trainium2 Bass kernel for nn_AttentionLayer_41188736368660.

Reference math (B=16, S=8192, D_MODEL=K_CH=OUT=256):
    q   = query @ Wq + bq                       # [B, OUT]
    k   = key @ Wk + bk                         # [B, S, OUT]
    v   = value @ Wv + bv                       # [B, S, OUT]
    s   = (q . k_s) / sqrt(OUT)                 # [B, S]
    w   = softmax(s)                            # [B, S]
    ctx = w @ v                                 # [B, OUT]
    out = broadcast ctx over S                  # [B, S, OUT]

Algebraic restructuring (exact):
    q . (key_s @ Wk + bk) = key_s . (Wk @ q) + const   (const cancels in softmax)
    w @ (value @ Wv + bv) = (w @ value) @ Wv + bv      (sum w = 1)
so the device only streams key and value once:
    s_s = key_s . qk,  e = exp(s),  T = sum e,  u = (e @ value) / T
with qk = Wk @ (query @ Wq + bq) * scale computed on host (tiny).

Device architecture (fp8, PE-centric):
  * key and value are cast to fp8-e4m3 on the host (whole-pipeline rel err
    vs the fp32 reference is ~9e-3, well under the 2e-2 gate; the HBM
    stream -- the roofline limit -- shrinks 4x vs fp32).
  * key is stored CHANNEL-major ([c, s] per batch), so the score
    dot-products run on the PE array: lhsT = qk half [128c, 32 replicas],
    rhs = key tile [128c, 512 positions] -> scores [32 replicas, 512] in
    PSUM, accumulated over the two 128-channel halves.  4 such groups pack
    one PSUM bank at partition offsets 0/32/64/96 (legal tile_position),
    so one ACT exp instruction covers a whole bank (2048 positions).
  * the 4 banks' exps of a batch land in ONE SBUF tile; a single
    flatten-scatter DMA per batch (128 descriptors of 128B -- descriptor
    count is what the DGE ring pays for) redistributes them to [128, 64]:
    each partition gets the weights of its 64 value rows.  Value is
    permuted on the host to match the scatter's stream order.
  * value carries a 257th all-ones channel, so the same PE accumulation
    that produces u also produces T = sum(w) -- no separate row-sum
    reduction or [128,x] store.
  * weighted value sum runs on PE: M=1 matmuls accumulating into 4 PSUM
    column strips (concurrent array column-strips).
  * host does the final tiny merge, 1/T normalize, Wv projection,
    broadcast.

Sharding: data-parallel over batch, B=16 -> 2 batches per core x 8 cores,
no cross-core communication.
"""

import numpy as np
import ml_dtypes

import concourse.bass as bass
import concourse.tile as tile
from concourse import mybir
from concourse.bass_utils import run_bass_kernel_spmd

B, S, C = 16, 8192, 256
CP = C + 1           # value channels + ones column (T accumulator)
N_CORES = 8
BPC = B // N_CORES   # batches per core
P = 128
NB = 4               # score PSUM banks per batch (2048 positions each)
NG = 4               # score groups per bank (one per 32-partition block)
GN = 512             # positions per score matmul (one PSUM bank row)
NCH = 64             # value chunks per batch (128 positions each)
MREP = 32            # qk replication in the score lhsT
SCALE = 1.0 / 16.0
F32 = mybir.dt.float32
BF16 = mybir.dt.bfloat16
F8 = mybir.dt.float8e4

FP8NP = ml_dtypes.float8_e4m3
BF16NP = ml_dtypes.bfloat16

_NC = None

# position of value row (p, j):  s = 2048*bank + 512*a + 64*r + j
# with a = p//32, bank = (p%32)//8, r = p%8  -- this matches the stream
# order of the per-batch scatter DMA (see _build_nc).
_PIDX = np.arange(P)
_POS = (2048 * ((_PIDX % 32) // 8) + 512 * (_PIDX // 32) + 64 * (_PIDX % 8))[
    :, None
] + np.arange(NCH)[None, :]


def _build_nc(split_waits=True):
    nc = bass.Bass("TRN2", target_bir_lowering=False, debug=False)

    # [b, c-half, c_local, s] channel-major key
    keyt_d = nc.dram_tensor("keyt", [BPC, 2, P, S], F8, kind="ExternalInput")
    # [b, p, j, c] value permuted to scatter order, plus ones channel
    valp_d = nc.dram_tensor("valp", [BPC, P, NCH, CP], F8, kind="ExternalInput")
    # [c_local, b, c-half, replica]
    qkb_d = nc.dram_tensor("qkb", [P, BPC, 2, MREP], BF16, kind="ExternalInput")
    u_d = nc.dram_tensor("u", [BPC, 4 * CP], F32, kind="ExternalOutput")

    # key tiles [128c, 4096s]; value tiles [128p, 32*CP]
    keyt_v = keyt_d.ap().rearrange("b h c (sh s) -> b h sh c s", sh=2)
    valp_v = valp_d.ap().rearrange("b p (jq j) c -> b jq p (j c)", jq=4)
    qkb_v = qkb_d.ap().rearrange("p b h m -> p (b h m)")

    with tile.TileContext(nc) as tc:
        with (
            tc.tile_pool(name="kpool", bufs=8) as kpool,
            tc.tile_pool(name="vpool", bufs=8) as vpool,
            tc.tile_pool(name="wpool", bufs=2) as wpool,
            tc.tile_pool(name="wcpool", bufs=2) as wcpool,
            tc.tile_pool(name="cpool", bufs=1) as cpool,
            tc.tile_pool(name="spool", bufs=4, space="PSUM") as spool,
            tc.tile_pool(name="apool", bufs=2, space="PSUM") as apool,
        ):
            # First key tile before the tiny constant load: the big stream
            # (SP ring) starts immediately; small DMAs (qkb, scatters,
            # stores) ride the ACT ring and never stall the stream.
            kt00 = kpool.tile([P, S // 2], F8, tag="kt")
            nc.sync.dma_start(out=kt00[:], in_=keyt_v[0, 0, 0])

            qkb_t = cpool.tile([P, BPC * 2 * MREP], BF16, tag="qkb")
            nc.scalar.dma_start(out=qkb_t[:], in_=qkb_v)

            kts = {}
            vts = {}

            def load_k(b, ch, sh):
                if (b, ch, sh) == (0, 0, 0):
                    kts[(b, ch, sh)] = kt00
                    return
                t = kpool.tile([P, S // 2], F8, tag="kt")
                nc.sync.dma_start(out=t[:], in_=keyt_v[b, ch, sh])
                kts[(b, ch, sh)] = t

            def load_v(b, jq):
                t = vpool.tile([P, (NCH // 4) * CP], F8, tag="vt")
                nc.sync.dma_start(out=t[:], in_=valp_v[b, jq])
                vts[(b, jq)] = t

            # SP ring order: both batches' keys stream before the late
            # value tiles, so all score banks + exp/scatter chains finish
            # mid-stream and each value block starts right as its tile
            # lands.  The last value tile's matmuls are the only
            # post-stream work.
            load_k(0, 0, 0); load_k(0, 1, 0); load_k(0, 0, 1); load_k(0, 1, 1)
            load_k(1, 0, 0); load_k(1, 1, 0); load_k(1, 0, 1); load_k(1, 1, 1)
            for b in range(BPC):
                for jq in range(4):
                    load_v(b, jq)

            wx_all = {}
            wcol_all = {}
            u_ps = {}
            u4 = {}

            def scores_bank(b, bank):
                """8 matmuls -> one exp into the batch's wx tile."""
                if bank == 0:
                    wx_t = wpool.tile([P, NB * GN], BF16, tag="wx")
                    wx_all[b] = wx_t
                h2, q = bank // 2, bank % 2
                ps = spool.tile([P, GN], F32, tag="ps")
                for ch in range(2):
                    lhsT = qkb_t[:, (b * 2 + ch) * MREP:(b * 2 + ch + 1) * MREP]
                    kt = kts[(b, ch, h2)]
                    for g in range(NG):
                        lo = q * (NG * GN) + g * GN
                        nc.tensor.matmul(
                            out=ps[g * MREP:(g + 1) * MREP, :],
                            lhsT=lhsT,
                            rhs=kt[:, lo:lo + GN],
                            start=(ch == 0),
                            stop=(ch == 1),
                            tile_position=(0, g * MREP),
                            skip_group_check=True,
                        )
                nc.scalar.activation(
                    out=wx_all[b][:, bank * GN:(bank + 1) * GN], in_=ps[:],
                    func=mybir.ActivationFunctionType.Exp,
                )

            def scatter_batch(b):
                """One DMA: wx rows {0,32,64,96} x (bank, n) -> [128, 64]."""
                wxa = wx_all[b][:]
                src = type(wxa)(
                    tensor=wxa.tensor,
                    offset=wxa.offset,
                    ap=[[MREP * NB * GN, NG], [GN, NB], [1, GN]],
                )
                wcol = wcpool.tile([P, NCH], BF16, tag="wcol")
                nc.scalar.dma_start(out=wcol[:], in_=src)
                wcol_all[b] = wcol

            def values_quarter(b, jq):
                wcol = wcol_all[b]
                vt = vts[(b, jq)]
                for jj in range(NCH // 4):
                    j = jq * (NCH // 4) + jj
                    g4 = j % 4
                    nc.tensor.matmul(
                        out=u_ps[b][g4 * 32:g4 * 32 + 1, :],
                        lhsT=wcol[:, j:j + 1],
                        rhs=vt[:, jj * CP:(jj + 1) * CP],
                        start=(j < 4),
                        stop=(j >= NCH - 4),
                        tile_position=(0, g4 * 32),
                        skip_group_check=True,
                    )

            def finish_batch(b):
                # one whole-tile PSUM->SBUF copy (lanes parallel, only the
                # 4 strip rows matter), then a 4-descriptor strided store.
                u4_t = cpool.tile([P, CP], F32, tag=f"u4{b}")
                u4[b] = u4_t
                nc.vector.tensor_copy(u4_t[:], u_ps[b][:])
                ua = u4_t[:]
                src4 = type(ua)(
                    tensor=ua.tensor,
                    offset=ua.offset,
                    ap=[[32 * CP, 4], [1, CP]],
                )
                nc.scalar.dma_start(out=u_d.ap()[b:b + 1, :], in_=src4)

            for b in range(BPC):
                ups_t = apool.tile([P, CP], F32, tag=f"ups{b}")
                u_ps[b] = ups_t

            # PE emission order is monotone with data arrival: each batch's
            # scores, then its value blocks; the PSUM->SBUF strip copies
            # and stores form one short uniform tail at the end.
            for bank in range(NB):
                scores_bank(0, bank)
            scatter_batch(0)
            for bank in range(NB):
                scores_bank(1, bank)
            scatter_batch(1)
            for jq in range(4):
                values_quarter(0, jq)
            finish_batch(0)
            for jq in range(4):
                values_quarter(1, jq)
            finish_batch(1)



    if split_waits:
        _split_multi_waits(nc)
    return nc


def _split_multi_waits(nc, max_waits=1):
    """Walrus encodes at most one sync-wait per TPB instruction ("Too many
    sync wait commands").  Hoist extra waits onto standalone EventSemaphore
    instructions inserted immediately before, on the same engine stream --
    semantically identical, no reordering."""
    n_split = 0
    for f in nc.m.functions:
        for blk in f.blocks:
            il = blk.instructions
            i = 0
            while i < len(il):
                inst = il[i]
                si = inst.sync_info
                if si is not None and len(si.on_wait) > max_waits:
                    waits = list(si.on_wait)
                    extra, keep = waits[:-max_waits], waits[-max_waits:]
                    for k, w in enumerate(extra):
                        ev = mybir.InstEventSemaphore(
                            name=f"{inst.name}-wsplit{k}",
                            engine=inst.engine,
                            ins=[],
                            outs=[],
                            sync_info=mybir.SyncInfo(on_wait=[w], on_update=[]),
                        )
                        il.insert(i, ev)
                        i += 1
                        n_split += 1
                    inst.sync_info = mybir.SyncInfo(
                        on_wait=keep, on_update=list(si.on_update)
                    )
                i += 1
    return n_split


def get_nc():
    global _NC
    if _NC is None:
        _NC = _build_nc()
    return _NC


def host_pre(query, Wq, bq, Wk):
    q = query @ Wq + bq          # [B, OUT]
    qk = q @ Wk.T                # [B, K_CH]
    return (qk * SCALE).astype(np.float32)


def make_in_maps(key, value, qk):
    """Per-core input maps for run_bass_kernel_spmd."""
    # qkb: [c_local, b, c-half, replica] bf16
    qkh = qk.reshape(B, 2, P).transpose(2, 0, 1)            # [128, B, 2]
    qkb = np.ascontiguousarray(
        np.broadcast_to(qkh[:, :, :, None], (P, B, 2, MREP))
    ).astype(BF16NP)
    in_maps = []
    for c in range(N_CORES):
        sl = slice(c * BPC, (c + 1) * BPC)
        keyt = np.ascontiguousarray(
            key[sl].transpose(0, 2, 1)
        ).reshape(BPC, 2, P, S).astype(FP8NP)
        # value rows permuted into scatter-stream order + ones channel
        vperm = value[sl][:, _POS.reshape(-1), :].reshape(BPC, P, NCH, C)
        valp = np.empty((BPC, P, NCH, CP), dtype=FP8NP)
        valp[..., :C] = vperm.astype(FP8NP)
        valp[..., C] = FP8NP(1.0)
        in_maps.append(
            {
                "keyt": keyt,
                "valp": valp,
                "qkb": np.ascontiguousarray(qkb[:, sl]),
            }
        )
    return in_maps


def host_post(u, Wv, bv):
    ctx = (u @ Wv + bv).astype(np.float32)   # [B, OUT]
    return np.broadcast_to(ctx[:, None, :], (B, S, C))


def kernel(query, key, value, Wq, bq, Wk, bk, Wv, bv, _results=None, _run_kwargs=None):
    query = np.asarray(query, np.float32)
    key = np.asarray(key, np.float32)
    value = np.asarray(value, np.float32)
    Wq = np.asarray(Wq, np.float32)
    bq = np.asarray(bq, np.float32)
    Wk = np.asarray(Wk, np.float32)
    Wv = np.asarray(Wv, np.float32)
    bv = np.asarray(bv, np.float32)

    qk = host_pre(query, Wq, bq, Wk)
    nc = get_nc()
    in_maps = make_in_maps(key, value, qk)
    res = run_bass_kernel_spmd(
        nc, in_maps, list(range(N_CORES)), **(_run_kwargs or {})
    )
    if _results is not None:
        _results.append(res)
    us = []
    for c in range(N_CORES):
        u4 = res.results[c]["u"].reshape(BPC, 4, CP).sum(axis=1)
        us.append(u4[:, :C] / u4[:, C:])
    u = np.concatenate(us, axis=0)
    return host_post(u, Wv, bv)
